# revision 1
# baseline (speedup 1.0000x reference)
"""CrossAttention TRN2 kernel v2: 8 cores = (batch 4) x (head-group 2).

Per core (batch b, 8 heads):
  all matmul operands fp16 (1 cyc/row); K pre-normalized on device with the
  norm reciprocal broadcast via selector matmuls, and the exp-approx scale
  LAM folded in so scores arrive in PSUM pre-scaled; softmax numerator
  p = exp(s)*exp(bias): 1024-wide ACT exp (scale=1/LAM) + the exp(bias)
  tensor-tensor multiply load-balanced across DVE and Pool; k/v/q
  projections stream through the shared 'ss' PSUM ring as queued work
  items inside the attention loop so only k(blk0)+q(blk0) gate the
  first exp;
  P@V uses the o[q,d] layout (moving dim 65 incl. an ones column that yields
  the softmax denominator), divided via DVE recip + tensor-scalar, then
  PE-transposed to o^T for the output projection. Host sums the two
  head-group partials per batch + p_bias.
"""
import sys
if '/opt/trn_rl_repo' not in sys.path:
    sys.path.insert(0, '/opt/trn_rl_repo')
import math
import numpy as np

B, L, IN = 4, 2048, 1024
H, DH = 16, 64
HG = 8                  # heads per core
EG = HG * DH            # embed per core = 512
NBLK = 4                # dh-blocks per core (2 heads each)
NQC = 4                 # q chunks of 512
NTC = 16                # t' chunks of 128
NKC = 8                 # contraction chunks of 128
EPS = 1e-12
MAXSM = math.log(100.0)

# exp(x) ~ (sq(v + C1E) + C2E)^16 with v = LAM*x, fitted on |x| <= 4.2
A2F = 0.5029542818006579
A1F = 1.0082434313766906
A0F = 0.9999247073952992
KAP = math.sqrt(A2F)
C1E = A1F / (2 * KAP)
C2E = A0F - A1F * A1F / (4 * A2F)
LAM = KAP / 16.0
S_ACT = 1.0 / LAM
INV_LAM2 = S_ACT * S_ACT

# routing knobs
MP_PRED = lambda cp, h: False   # Pool mult loses to DVE on the critical path
MP_NUM = 94             # of 256 p-tiles (1024-wide): multiply on Pool (rest DVE)
PV_LAG = 3  # cp-lag between p-tile production and P@V consumption

_CACHE = {}
DEBUG = False


def _ensure_expsq_op():
    from concourse import dve_ops
    from concourse.dve_spec import Spec, Src0, Src1, C0, C1, sq
    from concourse.dve_ops import DveOp
    name = "EXPSQ16_MUL_ANT"
    for op in dve_ops.OPS:
        if op.name == name:
            return op

    def ref(in0, in1, s0, s1, imm2):
        q = np.square(np.asarray(in0, np.float32) + np.float32(s0)) + np.float32(s1)
        for _ in range(4):
            q = np.square(q)
        return (q * np.asarray(in1, np.float32)).astype(np.float32)

    spec = Spec(body=sq(sq(sq(sq(sq(Src0 + C0) + C1)))) * Src1, reference=ref)
    op = DveOp(name, spec, subdim=False, uops_sha={})
    row = dve_ops._CUSTOM_DVE_ROW_BASE + len(dve_ops.OPS)
    assert row < 0x20
    dve_ops.OPS.append(op)
    dve_ops._SUB_OPCODE_FOR_NAME[name] = row
    dve_ops.CUSTOM_DVE_SPECS[name] = spec
    # self-pin the sha: compile() raises carrying the freshly computed value
    for ver in ("v3", "v4"):
        try:
            try:
                op.compile(ver)
            except ValueError as e:
                msg = str(e)
                got = msg.split(f"({ver}: ")[1].split(" ")[0]
                op.uops_sha[ver] = got
                op.compile(ver)
        except Exception:
            pass
    return op


def _build_nc():
    import concourse.bass as bass
    import concourse.mybir as mybir
    import concourse.tile as tile

    f32 = mybir.dt.float32
    f32r = mybir.dt.float32r
    f16 = mybir.dt.float16
    A = mybir.AluOpType
    AF = mybir.ActivationFunctionType

    nc = bass.Bass()
    dt_in = [
        ("xT", [IN, L], f16), ("yT", [IN, L], f16),
        ("WqT", [IN, EG], f16), ("WkT", [IN, EG], f16), ("WvT", [IN, EG], f16),
        ("WpT", [EG, IN], f16), ("qb", [128, NBLK], f32),
        ("smp2", [2, NBLK], f32), ("eBT", [NQC, L, 512], f16),
        ("sel", [32, NTC, 128], f16), ("E2", [128, 2], f16), ("E2T", [2, 128], f16),
        ("ident", [128, 128], f32),
    ]
    d = {n: nc.dram_tensor(n, s, t, kind="ExternalInput") for n, s, t in dt_in}
    out_d = nc.dram_tensor("out", [L, IN], f16, kind="ExternalOutput")
    if DEBUG:
        dbg_kn = nc.dram_tensor("dbg_kn", [128, NBLK, L], f16, kind="ExternalOutput")
        dbg_qn = nc.dram_tensor("dbg_qn", [128, NBLK, L], f16, kind="ExternalOutput")
        dbg_on = nc.dram_tensor("dbg_on", [NQC, 128, NBLK, 512], f32, kind="ExternalOutput")
        dbg_v = nc.dram_tensor("dbg_v", [128, NTC, HG * (DH + 1)], f16, kind="ExternalOutput")

    with tile.TileContext(nc) as tc:
        with tc.tile_pool(name="persist", bufs=1) as pp, \
             tc.tile_pool(name="ring2", bufs=2) as r2, \
             tc.tile_pool(name="ring3", bufs=2) as r3:
            kn = pp.tile([128, NBLK, L], f16, name="kn")
            qn = pp.tile([128, NBLK, L], f16, name="qn")
            vsb = pp.tile([128, NTC, HG * (DH + 1)], f16, name="vsb")
            WpTs = pp.tile([128, NBLK, IN], f16, name="WpTs")
            WqTs = pp.tile([128, NKC, EG], f16, name="WqTs")
            E2_s = pp.tile([128, 2], f16, name="E2_s")
            E2T_s = pp.tile([2, 128], f16, name="E2T_s")
            sel_s = pp.tile([32, NTC, 128], f16, name="sel_s")
            ident_s = pp.tile([128, 128], f32r, name="ident_s")
            ident2_s = pp.tile([128, 128], f32, name="ident2_s")
            qb_s = pp.tile([128, NBLK], f32, name="qb_s")
            smp2_s = pp.tile([2, NBLK], f32, name="smp2_s")
            nrmT = pp.tile([32, NBLK, 128], f16, name="nrmT")

            nc.scalar.dma_start(E2_s[:], d["E2"][:])
            nc.scalar.dma_start(E2T_s[:], d["E2T"][:])
            nc.scalar.dma_start(sel_s[:], d["sel"][:])
            nc.gpsimd.dma_start(ident_s[:], d["ident"][:])
            nc.scalar.dma_start(ident2_s[:], d["ident"][:])
            nc.scalar.dma_start(qb_s[:], d["qb"][:])
            nc.scalar.dma_start(smp2_s[:], d["smp2"][:])
            nc.scalar.dma_start(WpTs[:], d["WpT"].rearrange("(o p) c -> p o c", p=128))
            nc.scalar.dma_start(WqTs[:], d["WqT"].rearrange("(o p) e -> p o e", p=128))
            vr = vsb.rearrange("p t (s e) -> p t s e", e=DH + 1)
            nc.vector.memset(vr[:, :, :, DH], 1.0)

            # ---- streamed phases: k/v/q projections feed the attention loop ----
            with tc.tile_pool(name="atw", bufs=2) as atw, \
                 tc.tile_pool(name="pTp", bufs=3) as pTp, \
                 tc.tile_pool(name="eBp", bufs=2) as eBp, \
                 tc.tile_pool(name="lp", bufs=8) as lp, \
                 tc.tile_pool(name="pss", bufs=3, space="PSUM") as pss, \
                 tc.tile_pool(name="po", bufs=1, space="PSUM") as po:

                xTr = d["xT"].rearrange("(o p) t -> p o t", p=128)
                pools = {}
                pending_ops = []
                pending_div = []
                pending_qp = []
                pending_pv = []
                pending_misc = []

                def _pop(q, k):
                    for _ in range(min(k, len(q))):
                        q.pop(0)()

                def make_op_group(o_nT_t, qc, q1):
                    def fn():
                        ps = pss.tile([128, 1024], f32, name="ops", tag="ss")
                        for cs in range(2):
                            for eb in range(NBLK):
                                nc.tensor.matmul(
                                    ps[:, cs * 512:(cs + 1) * 512],
                                    o_nT_t[:, eb, q1 * 128:(q1 + 1) * 128],
                                    WpTs[:, eb, cs * 512:(cs + 1) * 512],
                                    start=(eb == 0), stop=(eb == NBLK - 1))
                            ob = r3.tile([128, 512], f16, name="ob", tag="ob")
                            nc.vector.tensor_copy(ob[:], ps[:, cs * 512:(cs + 1) * 512])
                            nc.sync.dma_start(
                                out_d[qc * 512 + q1 * 128: qc * 512 + (q1 + 1) * 128,
                                      cs * 512:(cs + 1) * 512], ob[:])
                    return fn

                def make_qproj(xq, qc, blk):
                    def fn():
                        qt = pss.tile([128, 1024], f32, name="qt", tag="ss")
                        for kk in range(NKC):
                            nc.tensor.matmul(
                                qt[:, 0:512], WqTs[:, kk, blk * 128:(blk + 1) * 128],
                                xq[:, kk, :], start=(kk == 0), stop=(kk == NKC - 1))
                        qTc = atw.tile([128, 512], f16, name="qTc", tag="qTc", bufs=1)
                        nc.vector.tensor_scalar(
                            qTc[:], qt[:, 0:512], qb_s[:, blk:blk + 1], None, A.add)
                        qsq = atw.tile([128, 512], f16, name="qsq", tag="qsq", bufs=1)
                        nc.gpsimd.tensor_tensor(qsq[:], qTc[:], qTc[:], A.mult)
                        nc.tensor.matmul(
                            qt[0:2, 512:1024], E2_s[:], qsq[:], start=True, stop=True)
                        bi = atw.tile([2, 512], f16, name="bi", tag="bi")
                        nc.scalar.activation(
                            bi[:], qt[0:2, 512:1024], AF.Sqrt, bias=0.0,
                            scale=smp2_s[:, blk:blk + 1])
                        nc.vector.tensor_scalar(bi[:], bi[:], EPS, None, A.max)
                        bir = atw.tile([2, 512], f16, name="bir", tag="bir")
                        with nc.allow_low_precision(reason="q norm recip"):
                            nc.vector.reciprocal(bir[:], bi[:])
                        nc.tensor.matmul(
                            qt[:, 0:512], E2T_s[:], bir[:], start=True, stop=True)
                        nc.vector.tensor_tensor(
                            qn[:, blk, qc * 512:(qc + 1) * 512], qTc[:], qt[:, 0:512],
                            A.mult)
                    return fn

                def run_qc(qc):
                    # queue q projection for the next chunk
                    if qc + 1 < NQC:
                        xq = atw.tile([128, NKC, 512], f16, name="xq", tag="xq")
                        nc.gpsimd.dma_start(
                            xq[:], xTr[:, :, (qc + 1) * 512:(qc + 2) * 512])
                        for blk in range(NBLK):
                            pending_qp.append(make_qproj(xq, qc + 1, blk))
                    eBh = []
                    for hf in range(2):
                        ebt = eBp.tile([128, NTC // 2, 512], f16, name="eBt", tag="eB",
                                       bufs=3)
                        nc.sync.dma_start(
                            ebt[:], d["eBT"][qc].rearrange("(c p) q -> p c q", p=128)
                            [:, hf * 8:(hf + 1) * 8, :])
                        eBh.append(ebt)
                    o_n = r2.tile([128, 4, 512], f32, name="o_n", tag="o_n")
                    for pair in range(NBLK):
                        oh = [po.tile([128, 4, DH + 1], f32, name=f"oh{h}", tag=f"oh{h}")
                              for h in range(2)]
                        pts = {}

                        def emit_pv(cp, oh=None, pts=None, pair=None):
                            for h in range(2):
                                pt = pts.pop((cp, h))
                                for sub in range(2):
                                    c = 2 * cp + sub
                                    for q1 in range(4):
                                        nc.tensor.matmul(
                                            oh[h][:, q1, :],
                                            pt[:, sub * 512 + q1 * 128:
                                               sub * 512 + (q1 + 1) * 128],
                                            vr[:, c, pair * 2 + h, :],
                                            start=(c == 0 and q1 == 0),
                                            stop=(c == NTC - 1),
                                            skip_group_check=True)

                        def make_div(oh, o_n, pair, h, q1):
                            def fn():
                                linv = lp.tile([128, 1], f32, name="linv", tag="linv")
                                nc.vector.reciprocal(linv[:], oh[h][:, q1, DH:DH + 1])
                                nc.vector.tensor_scalar(
                                    o_n[:, q1,
                                        (pair * 2 + h) * DH:(pair * 2 + h + 1) * DH],
                                    oh[h][:, q1, 0:DH], linv[:], None, A.mult)
                            return fn

                        for cp in range(NTC // 2):
                            _pop(pending_misc, 2)
                            if cp == 0:
                                _pop(pending_pv, 2)
                            elif cp == 1:
                                _pop(pending_pv, 1)
                                _pop(pending_div, 4)
                            elif cp == 2:
                                _pop(pending_div, 4)
                            elif cp == 4:
                                _pop(pending_ops, 1)
                            elif cp == 5:
                                _pop(pending_qp, 1)
                            elif cp == 6:
                                _pop(pending_ops, 1)
                            for h in range(2):
                                ss = pss.tile([128, 1024], f32, name="ss", tag="ss")
                                for sub in range(2):
                                    c = 2 * cp + sub
                                    nc.tensor.matmul(
                                        ss[:, sub * 512:(sub + 1) * 512],
                                        kn[h * 64:(h + 1) * 64, pair, c * 128:(c + 1) * 128],
                                        qn[h * 64:(h + 1) * 64, pair, qc * 512:(qc + 1) * 512],
                                        start=True, stop=True)
                                pt = pTp.tile([128, 1024], f16, name=f"pT{h}", tag=f"pT{h}")
                                nc.scalar.activation(
                                    pt[:], ss[:], AF.Exp, bias=0.0, scale=S_ACT)
                                eng = nc.gpsimd if MP_PRED(cp, h) else nc.vector
                                eng.tensor_tensor(
                                    pt[:], pt[:],
                                    eBh[cp // 4][:, (2 * cp) % 8:(2 * cp) % 8 + 2, :]
                                    .rearrange("p a q -> p (a q)"), A.mult)
                                pts[(cp, h)] = pt
                            if cp >= PV_LAG:
                                emit_pv(cp - PV_LAG, oh=oh, pts=pts, pair=pair)
                        for cp in range(NTC // 2 - PV_LAG, NTC // 2):
                            pending_pv.append(
                                (lambda c=cp, o=oh, p=pts, pr=pair:
                                 emit_pv(c, oh=o, pts=p, pair=pr)))
                        for h in range(2):
                            for q1 in range(4):
                                pending_div.append(make_div(oh, o_n, pair, h, q1))
                    return o_n

                def finish_qc(qc, o_n):
                    o_nT = r2.tile([128, NBLK, 512], f16, name="o_nT", tag="o_nT")
                    for qp2 in range(2):
                        tpt = pss.tile([128, 1024], f32, name="tpt", tag="ss")
                        tpv = tpt.rearrange("p (a e) -> p a e", e=128)
                        for half in range(2):
                            q1 = qp2 * 2 + half
                            for eb in range(NBLK):
                                nc.tensor.matmul(
                                    tpv[:, half * 4 + eb, :],
                                    o_n[:, q1, eb * 128:(eb + 1) * 128],
                                    ident2_s[:], is_transpose=True,
                                    start=(eb == 0), stop=True,
                                    skip_group_check=True)
                            nc.vector.tensor_copy(
                                o_nT[:, :, q1 * 128:(q1 + 1) * 128],
                                tpv[:, half * 4:half * 4 + 4, :])
                    for q1 in range(4):
                        pending_ops.append(make_op_group(o_nT, qc, q1))

                # ---- qc0 with streamed k/v projections ----
                with tc.tile_pool(name="ph1", bufs=1) as ph1, \
                     tc.tile_pool(name="ph1w", bufs=1) as ph1w:
                    yTs = ph1.tile([128, NKC, L], f16, name="yTs")
                    for t5 in range(4):
                        nc.gpsimd.dma_start(
                            yTs[:, :, t5 * 512:(t5 + 1) * 512],
                            d["yT"].rearrange("(o p) t -> p o t", p=128)
                            [:, :, t5 * 512:(t5 + 1) * 512])
                    WkTs = ph1.tile([128, NKC, EG], f16, name="WkTs")
                    nc.sync.dma_start(WkTs[:], d["WkT"].rearrange("(o p) e -> p o e", p=128))
                    WvTs = ph1.tile([128, NKC, EG], f16, name="WvTs")
                    nc.scalar.dma_start(WvTs[:], d["WvT"].rearrange("(o p) e -> p o e", p=128))

                    def make_kproj(blk, half, ksq):
                        def fn():
                            ps = pss.tile([128, 1024], f32, name="kps", tag="ss")
                            for t5h in range(2):
                                t5 = half * 2 + t5h
                                for kk in range(NKC):
                                    nc.tensor.matmul(
                                        ps[:, t5h * 512:(t5h + 1) * 512],
                                        WkTs[:, kk, blk * 128:(blk + 1) * 128],
                                        yTs[:, kk, t5 * 512:(t5 + 1) * 512],
                                        start=(kk == 0), stop=(kk == NKC - 1))
                                nc.vector.tensor_copy(
                                    kn[:, blk, t5 * 512:(t5 + 1) * 512],
                                    ps[:, t5h * 512:(t5h + 1) * 512])
                                nc.gpsimd.tensor_tensor(
                                    ksq[:, t5 * 512:(t5 + 1) * 512],
                                    kn[:, blk, t5 * 512:(t5 + 1) * 512],
                                    kn[:, blk, t5 * 512:(t5 + 1) * 512], A.mult)
                        return fn

                    def make_knorm(blk, ksq):
                        def fn():
                            nt = pss.tile([128, 1024], f32, name="nt", tag="ss")
                            for c in range(NTC):
                                nc.tensor.matmul(
                                    nt[:, 2 * c:2 * c + 2], ksq[:, c * 128:(c + 1) * 128],
                                    E2_s[:], start=True, stop=True)
                            nrm = ph1w.tile([128, 2 * NTC], f32, name="nrm", tag="nrm")
                            nc.scalar.activation(nrm[:], nt[:, 0:32], AF.Sqrt,
                                                 bias=0.0, scale=INV_LAM2)
                            nc.vector.tensor_scalar(nrm[:], nrm[:], EPS * S_ACT, None, A.max)
                            nc.tensor.matmul(
                                nt[0:32, 32:160], nrm[:], ident2_s[:],
                                is_transpose=True, start=True, stop=True,
                                skip_group_check=True)
                            with nc.allow_low_precision(reason="k norm recip"):
                                nc.vector.reciprocal(nrmT[:, blk, :], nt[0:32, 32:160])
                            for cg in range(4):
                                ab = nt[:, 512:1024]
                                for ci in range(4):
                                    c = cg * 4 + ci
                                    nc.tensor.matmul(
                                        ab[:, ci * 128:(ci + 1) * 128], sel_s[:, c, :],
                                        nrmT[:, blk, :], start=True, stop=True)
                                nc.vector.tensor_tensor(
                                    kn[:, blk, cg * 512:(cg + 1) * 512],
                                    kn[:, blk, cg * 512:(cg + 1) * 512], ab[:], A.mult)
                        return fn

                    def make_vproj(cpair):
                        def fn():
                            ps = pss.tile([128, 1024], f32, name="vps", tag="ss")
                            for sub in range(2):
                                tb = 2 * cpair + sub
                                for kk in range(NKC):
                                    nc.tensor.matmul(
                                        ps[:, sub * 512:(sub + 1) * 512],
                                        yTs[:, kk, tb * 128:(tb + 1) * 128],
                                        WvTs[:, kk, :], start=(kk == 0), stop=(kk == NKC - 1))
                                nc.vector.tensor_copy(
                                    vr[:, tb, :, 0:DH],
                                    ps[:, sub * 512:(sub + 1) * 512].rearrange(
                                        "p (s e) -> p s e", e=DH))
                        return fn

                    # inline prefix: k(blk0) + its norm + q(qc0, blk0)
                    ksqs = {}
                    for blk in range(NBLK):
                        ksqs[blk] = None
                    def kitems(blk):
                        ksq = ph1w.tile([128, L], f16, name="ksq", tag="ksq",
                                        uniquify=True)
                        return [make_kproj(blk, 0, ksq), make_kproj(blk, 1, ksq),
                                make_knorm(blk, ksq)]

                    xq0 = atw.tile([128, NKC, 512], f16, name="xq", tag="xq")
                    nc.gpsimd.dma_start(xq0[:], xTr[:, :, 0:512])
                    for fn in kitems(0):
                        fn()
                    make_qproj(xq0, 0, 0)()

                    kb1 = kitems(1)
                    kb2 = kitems(2)
                    kb3 = kitems(3)
                    pending_misc.extend([
                        make_vproj(0), make_vproj(1), kb1[0], kb1[1],
                        make_vproj(2), make_vproj(3), kb1[2], make_qproj(xq0, 0, 1),
                        make_vproj(4), make_vproj(5), kb2[0], kb2[1],
                        make_vproj(6), make_vproj(7), kb2[2], make_qproj(xq0, 0, 2),
                        kb3[0], kb3[1], kb3[2], make_qproj(xq0, 0, 3),
                    ])
                    o_n0 = run_qc(0)
                    _pop(pending_pv, len(pending_pv))
                    _pop(pending_div, len(pending_div))
                finish_qc(0, o_n0)
                for qc in range(1, NQC):
                    o_nq = run_qc(qc)
                    _pop(pending_pv, len(pending_pv))
                    _pop(pending_div, len(pending_div))
                    finish_qc(qc, o_nq)
                _pop(pending_ops, len(pending_ops))
                if DEBUG:
                    nc.sync.dma_start(dbg_kn[:], kn[:])
                    nc.sync.dma_start(dbg_qn[:], qn[:])
                    nc.sync.dma_start(dbg_v[:], vsb[:])
    _split_excess_waits(nc)
    return nc


def _split_excess_waits(nc):
    import concourse.mybir as mybir
    for f in nc.m.functions:
        for bb in f.blocks:
            new_insts = []
            for inst in bb.instructions:
                si = inst.sync_info
                if si is not None and si.on_wait and len(si.on_wait) > 1:
                    waits = list(si.on_wait)
                    for ci, w in enumerate(waits[:-1]):
                        new_insts.append(mybir.InstNoOp(
                            name=f"{inst.name}-ws{ci}", engine=inst.engine,
                            ins=[], outs=[],
                            sync_info=mybir.SyncInfo(on_wait=[w], on_update=[])))
                    inst.sync_info = mybir.SyncInfo(
                        on_wait=waits[-1:], on_update=si.on_update)
                new_insts.append(inst)
            bb.instructions[:] = new_insts


def kernel(x, y, attn_bias, Wq, Wk, Wv, q_bias, scale_mul, Wp, p_bias):
    from concourse.bass_utils import run_bass_kernel_spmd
    if "nc" not in _CACHE:
        _CACHE["nc"] = _build_nc()
    nc = _CACHE["nc"]

    x = np.asarray(x, dtype=np.float32)
    y = np.asarray(y, dtype=np.float32)
    bias = np.asarray(attn_bias, dtype=np.float32)[0, 0]
    Wq = np.asarray(Wq, dtype=np.float32); Wk = np.asarray(Wk, dtype=np.float32)
    Wv = np.asarray(Wv, dtype=np.float32); Wp = np.asarray(Wp, dtype=np.float32)
    q_bias = np.asarray(q_bias, dtype=np.float32)
    p_bias = np.asarray(p_bias, dtype=np.float32)
    sm = np.exp(np.minimum(np.asarray(scale_mul, dtype=np.float32), MAXSM))[0, :, 0, 0]

    eBT = np.ascontiguousarray(
        np.exp(bias.T).reshape(L, NQC, 512).transpose(1, 0, 2)).astype(np.float16)
    E2 = np.zeros((128, 2), np.float16); E2[0:64, 0] = 1; E2[64:128, 1] = 1
    E2T = np.ascontiguousarray(E2.T)
    sel = np.zeros((32, NTC, 128), np.float16)
    for c in range(NTC):
        sel[2 * c, c, 0:64] = 1
        sel[2 * c + 1, c, 64:128] = 1
    ident = np.eye(128, dtype=np.float32)
    xT = [np.ascontiguousarray(x[b].T).astype(np.float16) for b in range(B)]
    yT = [np.ascontiguousarray(y[b].T).astype(np.float16) for b in range(B)]

    in_maps = []
    for core in range(8):
        b, g = core // 2, core % 2
        sl = slice(g * EG, (g + 1) * EG)
        smg = sm[g * HG:(g + 1) * HG].reshape(NBLK, 2).T     # [2, NBLK]
        smp2 = np.ascontiguousarray(1.0 / (smg * smg)).astype(np.float32)
        qb_t = np.ascontiguousarray(
            q_bias[sl].reshape(NBLK, 128).T).astype(np.float32)
        in_maps.append({
            "xT": xT[b], "yT": yT[b],
            "WqT": np.ascontiguousarray(Wq[sl].T).astype(np.float16),
            "WkT": np.ascontiguousarray(Wk[sl].T).astype(np.float16),
            "WvT": np.ascontiguousarray(Wv[sl].T).astype(np.float16),
            "WpT": np.ascontiguousarray(Wp[:, sl].T).astype(np.float16),
            "qb": qb_t, "smp2": smp2, "eBT": eBT,
            "sel": sel, "E2": E2, "E2T": E2T, "ident": ident,
        })
    res = run_bass_kernel_spmd(nc, in_maps, core_ids=list(range(8)))
    if DEBUG:
        _CACHE["dbg"] = res.results
    parts = [r["out"] for r in res.results]
    out = np.empty((B, L, IN), np.float32)
    for b in range(B):
        out[b] = parts[2 * b].astype(np.float32) + parts[2 * b + 1] + p_bias
    return out

